# revision 6
# baseline (speedup 1.0000x reference)
"""Causal self-attention kernel for TRN2 (8 NeuronCores, Bass/Tile).

Problem: B=8, T=1024, C=768, H=12, HD=64.
  qkv = x @ W_attn + b_attn ; causal softmax attention ; y = att_out @ W_proj + b_proj

Sharding: pure data-parallel over batch — core b computes batch element b.

Per-core dataflow (all matmuls in fp32r = full-rate reduced-precision fp32):
  xT   [768,1024]  = PE-transpose of x                      (lhsT/rhs source)
  qkT  [1536,1024] = (W_qk)^T-style projection: qkT[c',t] = sum_c W[c,c'] xT[c,t]
  V    [1024,768]  : V[t,c'] = sum_c xT[c,t] W_v[c,c']      (per-head Vp tiles with
                     a leading ones column -> PV matmul also produces Z row)
  per head h, i-block (512 cols):
     ST[j,i] = kT^T q  (K=64, causal-trimmed)   -> exp(0.125*ST) on ScalarE -> fp32r
     tri-mask on diagonal 128x128 sub-block (multiplicative, post-exp)
     OT'[0,:] = Z, OT'[1:65,:] = unnormalized attention out (transposed), accumulated
     ATn[c,t] = OT'[1:65]/Z  (DVE mult by gpsimd-broadcast 1/Z)
  y[t,:] = ATn^T-contraction with W_proj + b_proj
"""

import numpy as np

import concourse.bass as bass
import concourse.mybir as mybir
import concourse.tile as tile
from concourse import bacc
from concourse.bass_utils import run_bass_kernel_spmd

F32 = mybir.dt.float32
F32R = mybir.dt.float32r
AF = mybir.ActivationFunctionType
ALU = mybir.AluOpType

T, C, H, HD = 1024, 768, 12, 64
NCORES = 8
CC = C // 128          # 6 contraction chunks
TP = T // 128          # 8 t-chunks of 128
TB = T // 512          # 2 t-blocks of 512
QKCP = 2 * C // 128    # 12 qkT partition tiles
SCALE = 1.0 / 8.0      # 1/sqrt(64)

_PROGRAM_CACHE = {}


def build_program():
    nc = bacc.Bacc("TRN2", target_bir_lowering=False, debug=False)

    x_d = nc.dram_tensor("x", [T, C], F32, kind="ExternalInput").ap()
    wa_d = nc.dram_tensor("W_attn", [C, 3 * C], F32, kind="ExternalInput").ap()
    ba_d = nc.dram_tensor("b_attn", [1, 3 * C], F32, kind="ExternalInput").ap()
    wp_d = nc.dram_tensor("W_proj", [C, C], F32, kind="ExternalInput").ap()
    bp_d = nc.dram_tensor("b_proj", [1, C], F32, kind="ExternalInput").ap()
    y_d = nc.dram_tensor("y", [T, C], F32, kind="ExternalOutput").ap()

    with tile.TileContext(nc) as tc:
        _emit(nc, tc, x_d, wa_d, ba_d, wp_d, bp_d, y_d)
    nc.compile()
    return nc


def _emit(nc, tc, x_d, wa_d, ba_d, wp_d, bp_d, y_d):
    from contextlib import ExitStack

    ctx = ExitStack()
    with ctx:
        const_pool = ctx.enter_context(tc.tile_pool(name="consts", bufs=1))
        ps_work = ctx.enter_context(tc.tile_pool(name="ps_work", bufs=3, space="PSUM"))
        ps_acc = ctx.enter_context(tc.tile_pool(name="ps_acc", bufs=2, space="PSUM"))

        # ---- constants -------------------------------------------------
        ident = const_pool.tile([128, 128], F32, name="ident")
        nc.gpsimd.memset(ident[:], 0.0)
        nc.gpsimd.affine_select(
            out=ident[:], in_=ident[:], compare_op=ALU.not_equal, fill=1.0,
            base=0, pattern=[[-1, 128]], channel_multiplier=1,
        )
        # tri[j, i] = 1.0 if j <= i else 0.0   (keep lower-causal in [j,i] layout)
        tri = const_pool.tile([128, 128], F32, name="tri")
        nc.gpsimd.memset(tri[:], 1.0)
        nc.gpsimd.affine_select(
            out=tri[:], in_=tri[:], compare_op=ALU.is_ge, fill=0.0,
            base=0, pattern=[[1, 128]], channel_multiplier=-1,
        )
        ones32 = const_pool.tile([128, 16], F32, name="ones32")
        nc.gpsimd.memset(ones32[:], 1.0)
        ones_row_f32 = const_pool.tile([1, 512], F32, name="ones_row_f32")
        nc.gpsimd.memset(ones_row_f32[:], 1.0)
        ones_row = const_pool.tile([1, 512], F32R, name="ones_row")
        nc.vector.tensor_copy(ones_row[:], ones_row_f32[:])

        ba_sb = const_pool.tile([1, 3 * C], F32R, name="ba_sb")
        nc.gpsimd.dma_start(ba_sb[:], ba_d[:, :])
        bp_sb = const_pool.tile([1, C], F32R, name="bp_sb")
        nc.gpsimd.dma_start(bp_sb[:], bp_d[:, :])

        # ---- phase A: load x, build xT [768, 1024] ---------------------
        # phase-limited pools live on the RIGHT side of SBUF and are
        # released after phase B so attention-phase pools fit.
        phase_ctx = ExitStack()
        xt_pool = phase_ctx.enter_context(tc.tile_pool(name="xt", bufs=1, side="right"))
        xsb_pool = phase_ctx.enter_context(tc.tile_pool(name="xsb", bufs=3, side="right"))

        xT = []
        for cc in range(CC):
            t_ = xt_pool.tile([128, T], F32R, name=f"xT_{cc}", tag=f"xT{cc}")
            xT.append(t_)
        for tp in range(TP):
            x_sb = xsb_pool.tile([128, C], F32, name=f"x_sb_{tp}", tag="x_sb")
            nc.sync.dma_start(x_sb[:], x_d[tp * 128 : (tp + 1) * 128, :])
            for cc in range(CC):
                pt = ps_work.tile([128, 128], F32, name=f"ps_xt_{tp}_{cc}", tag="ps")
                nc.tensor.transpose(pt[:], x_sb[:, cc * 128 : (cc + 1) * 128], ident[:])
                nc.vector.tensor_copy(xT[cc][:, tp * 128 : (tp + 1) * 128], pt[:])

        # ---- phase B: W_attn load; qkT + Vp ----------------------------
        w_pool = phase_ctx.enter_context(tc.tile_pool(name="w", bufs=1, side="right"))
        W = []
        for cc in range(CC):
            w_t = w_pool.tile([128, 3 * C], F32R, name=f"W_{cc}", tag=f"W{cc}")
            nc.gpsimd.dma_start(w_t[:], wa_d[cc * 128 : (cc + 1) * 128, :])
            W.append(w_t)

        qkt_pool = ctx.enter_context(tc.tile_pool(name="qkt", bufs=1))
        qkT = []
        for cp in range(QKCP):
            t_ = qkt_pool.tile([128, T], F32R, name=f"qkT_{cp}", tag=f"qkT{cp}")
            qkT.append(t_)

        for cp in range(QKCP):
            for tb in range(TB):
                pq = ps_acc.tile([128, 512], F32, name=f"ps_qk_{cp}_{tb}", tag="acc")
                for cc in range(CC):
                    nc.tensor.matmul(
                        pq[:],
                        W[cc][:, cp * 128 : (cp + 1) * 128],
                        xT[cc][:, tb * 512 : (tb + 1) * 512],
                        start=(cc == 0),
                        stop=False,
                    )
                # + b_attn[c'] (per-partition): rank-1 with ones row
                nc.tensor.matmul(
                    pq[:],
                    ba_sb[:, cp * 128 : (cp + 1) * 128],
                    ones_row[:],
                    start=False,
                    stop=True,
                )
                nc.vector.tensor_copy(qkT[cp][:, tb * 512 : (tb + 1) * 512], pq[:])

        # Vp tiles: per t-chunk, [128, 12*65]; head h at cols [65h..65h+64],
        # col 65h is the ones column (-> Z row in the PV matmul output).
        vp_pool = ctx.enter_context(tc.tile_pool(name="vp", bufs=1))
        Vp = []
        for tp in range(TP):
            t_ = vp_pool.tile([128, H * 65], F32R, name=f"Vp_{tp}", tag=f"Vp{tp}")
            Vp.append(t_)
            nc.vector.tensor_copy(
                t_.rearrange("p (h e) -> p h e", e=65)[:, :, 64:65],
                ones32[:, 0:H].rearrange("p (h e) -> p h e", e=1),
            )

        for tp in range(TP):
            for vc in range(2):  # v cols [1536+384*vc : 1536+384*(vc+1)]
                pv = ps_acc.tile([128, 384], F32, name=f"ps_v_{tp}_{vc}", tag="acc")
                for cc in range(CC):
                    nc.tensor.matmul(
                        pv[:],
                        xT[cc][:, tp * 128 : (tp + 1) * 128],
                        W[cc][:, 1536 + vc * 384 : 1536 + (vc + 1) * 384],
                        start=(cc == 0),
                        stop=False,
                    )
                nc.tensor.matmul(
                    pv[:],
                    ones_row[:, 0:128],
                    ba_sb[:, 1536 + vc * 384 : 1536 + (vc + 1) * 384],
                    start=False,
                    stop=True,
                )
                for hh in range(6):  # heads 6*vc + hh
                    h = 6 * vc + hh
                    nc.vector.tensor_copy(
                        Vp[tp][:, h * 65 : h * 65 + 64],
                        pv[:, hh * 64 : (hh + 1) * 64],
                    )

        phase_ctx.close()  # release xt/xsb/w SBUF before attention pools

        # ---- W_proj prefetch ------------------------------------------
        wp_pool = ctx.enter_context(tc.tile_pool(name="wp", bufs=1))
        Wp = []
        for cc in range(CC):
            w_t = wp_pool.tile([128, C], F32R, name=f"Wp_{cc}", tag=f"Wp{cc}")
            nc.gpsimd.dma_start(w_t[:], wp_d[cc * 128 : (cc + 1) * 128, :])
            Wp.append(w_t)

        # ---- phase C: attention ---------------------------------------
        atn_pool = ctx.enter_context(tc.tile_pool(name="atn", bufs=1))
        ATn = []
        for cp in range(CC):
            t_ = atn_pool.tile([128, T], F32R, name=f"ATn_{cp}", tag=f"ATn{cp}")
            ATn.append(t_)

        est_pool = ctx.enter_context(tc.tile_pool(name="est", bufs=6))
        nrm_pool = ctx.enter_context(tc.tile_pool(name="nrm", bufs=3))

        for h in range(H):
            qt = qkT[h // 2]
            kt = qkT[6 + h // 2]
            r0 = (h % 2) * 64  # row offset of this head inside the 128-row tile
            for ib in range(TB):
                po = ps_acc.tile([65, 512], F32, name=f"ps_ot_{h}_{ib}", tag="acc")
                njc = 4 * (ib + 1)
                for jc in range(njc):
                    r = jc - 4 * ib
                    col0 = max(r, 0) * 128
                    nw = 512 - col0
                    pst = ps_work.tile([128, 512], F32, name=f"ps_st_{h}_{ib}_{jc}", tag="ps")
                    nc.tensor.matmul(
                        pst[:, col0:512],
                        kt[r0 : r0 + 64, jc * 128 : (jc + 1) * 128],
                        qt[r0 : r0 + 64, ib * 512 + col0 : (ib + 1) * 512],
                        start=True,
                        stop=True,
                    )
                    est = est_pool.tile([128, 512], F32R, name=f"est_{h}_{ib}_{jc}", tag="est")
                    nc.scalar.activation(est[:, col0:512], pst[:, col0:512], AF.Exp, scale=SCALE)
                    if r >= 0:
                        # mask the diagonal 128x128 sub-block (multiplicative)
                        nc.vector.tensor_tensor(
                            est[:, col0 : col0 + 128],
                            est[:, col0 : col0 + 128],
                            tri[:],
                            op=ALU.mult,
                        )
                    nc.tensor.matmul(
                        po[:, col0:512],
                        Vp[jc][:, h * 65 : h * 65 + 65],
                        est[:, col0:512],
                        start=(jc == 0),
                        stop=(jc == njc - 1),
                    )
                # normalization: ATn rows = OT'[1:65] / Z
                zinv = nrm_pool.tile([1, 512], F32, name=f"zinv_{h}_{ib}", tag="zinv")
                nc.vector.reciprocal(zinv[:], po[64:65, :])
                zb = nrm_pool.tile([64, 512], F32, name=f"zb_{h}_{ib}", tag="zb")
                nc.gpsimd.partition_broadcast(zb[:], zinv[:])
                nc.vector.tensor_tensor(
                    ATn[h // 2][r0 : r0 + 64, ib * 512 : (ib + 1) * 512],
                    po[0:64, :],
                    zb[:],
                    op=ALU.mult,
                )

        # ---- phase D: projection --------------------------------------
        y_pool = ctx.enter_context(tc.tile_pool(name="ysb", bufs=3))
        for tp in range(TP):
            y_sb = y_pool.tile([128, C], F32, name=f"y_sb_{tp}", tag="y_sb")
            for oc in range(2):
                py = ps_acc.tile([128, 384], F32, name=f"ps_y_{tp}_{oc}", tag="acc")
                for cp in range(CC):
                    nc.tensor.matmul(
                        py[:],
                        ATn[cp][:, tp * 128 : (tp + 1) * 128],
                        Wp[cp][:, oc * 384 : (oc + 1) * 384],
                        start=(cp == 0),
                        stop=False,
                    )
                nc.tensor.matmul(
                    py[:],
                    ones_row[:, 0:128],
                    bp_sb[:, oc * 384 : (oc + 1) * 384],
                    start=False,
                    stop=True,
                )
                nc.vector.tensor_copy(y_sb[:, oc * 384 : (oc + 1) * 384], py[:])
            nc.sync.dma_start(y_d[tp * 128 : (tp + 1) * 128, :], y_sb[:])


def kernel(x, W_attn, b_attn, W_proj, b_proj, _trace=False, _trace_kwargs=None):
    x = np.ascontiguousarray(np.asarray(x, dtype=np.float32))
    W_attn = np.ascontiguousarray(np.asarray(W_attn, dtype=np.float32))
    b_attn = np.ascontiguousarray(np.asarray(b_attn, dtype=np.float32)).reshape(1, 3 * C)
    W_proj = np.ascontiguousarray(np.asarray(W_proj, dtype=np.float32))
    b_proj = np.ascontiguousarray(np.asarray(b_proj, dtype=np.float32)).reshape(1, C)

    if "prog" not in _PROGRAM_CACHE:
        _PROGRAM_CACHE["prog"] = build_program()
    nc = _PROGRAM_CACHE["prog"]

    in_maps = [
        {
            "x": np.ascontiguousarray(x[b]),
            "W_attn": W_attn,
            "b_attn": b_attn,
            "W_proj": W_proj,
            "b_proj": b_proj,
        }
        for b in range(NCORES)
    ]
    res = run_bass_kernel_spmd(
        nc,
        in_maps,
        core_ids=list(range(NCORES)),
        trace=_trace,
        **(_trace_kwargs or {}),
    )
    out = np.stack([res.results[b]["y"] for b in range(NCORES)], axis=0)
    if _trace:
        return out, res
    return out


if __name__ == "__main__":
    rng = np.random.default_rng(0)
    x = rng.standard_normal((NCORES, T, C)).astype(np.float32)
    W_attn = (rng.standard_normal((C, 3 * C)) * 0.02).astype(np.float32)
    b_attn = np.zeros(3 * C, np.float32)
    W_proj = (rng.standard_normal((C, C)) * 0.02).astype(np.float32)
    b_proj = np.zeros(C, np.float32)
    y = kernel(x=x, W_attn=W_attn, b_attn=b_attn, W_proj=W_proj, b_proj=b_proj)
    print("out", y.shape, y.dtype, np.abs(y).max())
